# revision 14
# baseline (speedup 1.0000x reference)
"""TRN2 Bass kernel for nn_ReadHead (B=8192, C=1024, M=4096) on 8 NeuronCores.

Reference computation:
    key = softmax(controller_output @ W + b, axis=-1)            # [B, M]
    sims = -cos_sim(key, memory) per row                         # [B]
    rw = softmax(sims over the batch axis)                       # [B]
    out[i] = rw[i] * sum_j(memory[i, j])                         # [B]

Kernel strategy (per core, batch-sharded 1024 rows):
  - Inputs are shipped as bf16 (the memory-bound regime: halves HBM traffic;
    output error stays ~1e-3 of scale since all accumulation is fp32).
  - logits tile [128 rows, 512 cols] accumulated on PE in bf16 (lhsT =
    controller^T chunk, rhs = W chunk), 8 k-steps, fp32 PSUM.
  - ACT: e = exp(logits) and sum(e^2) = accum(exp(2*logits)) straight from
    PSUM (cosine sim is scale-invariant, so the softmax normalizer and the
    row max-subtraction cancel; logits are O(3) so exp is safe in fp32).
  - DVE: dot(e, mem) and sum(mem^2) via the custom TENSOR_TENSOR_REDUCE;
    sum(mem) alternates between ACT (Identity+accum) and DVE
    (tensor_scalar+accum) to balance the two engines.
  - sims = -dot * rsqrt(e2 * m2); exp(sims) via ACT; the batch softmax
    denominator is an AllReduce(add) of the per-core partial sums; finally
    out = exp(sims) * rowsum(mem) / S.
"""

import numpy as np

_B, _C, _M = 8192, 1024, 4096
_NC = 8
_BS = _B // _NC      # 1024 rows per core
_RT = _BS // 128     # 8 row tiles
_MC = _M // 512      # 8 column chunks
_KC = _C // 128      # 8 contraction chunks

_built_cache = {}


def _build(with_bias: bool):
    from contextlib import ExitStack

    import concourse.bass as bass
    import concourse.tile as tile
    from concourse import bacc, mybir
    from concourse.dve_ops import TENSOR_TENSOR_REDUCE

    f32 = mybir.dt.float32
    bf16 = mybir.dt.bfloat16
    AF = mybir.ActivationFunctionType
    OP = mybir.AluOpType

    nc = bacc.Bacc("TRN2", target_bir_lowering=False, debug=False, num_devices=_NC)

    ct_d = nc.dram_tensor("ct", [_C, _BS], bf16, kind="ExternalInput")
    w_d = nc.dram_tensor("w", [_C, _M], bf16, kind="ExternalInput")
    m_d = nc.dram_tensor("mem", [_BS, _M], bf16, kind="ExternalInput")
    if with_bias:
        b_d = nc.dram_tensor("bias", [1, _M], bf16, kind="ExternalInput")
    out_d = nc.dram_tensor("out", [_BS], f32, kind="ExternalOutput")

    with tile.TileContext(nc) as tc, ExitStack() as ctx:
        singles = ctx.enter_context(tc.tile_pool(name="singles", bufs=1))
        wpool = ctx.enter_context(tc.tile_pool(name="wpool", bufs=2))
        mpool = ctx.enter_context(tc.tile_pool(name="mpool", bufs=8))
        epool = ctx.enter_context(tc.tile_pool(name="epool", bufs=6))
        spool = ctx.enter_context(tc.tile_pool(name="spool", bufs=4))
        fin = ctx.enter_context(tc.tile_pool(name="fin", bufs=1))
        psum = ctx.enter_context(tc.tile_pool(name="psum", bufs=7, space="PSUM"))
        psum1 = ctx.enter_context(tc.tile_pool(name="psum1", bufs=1, space="PSUM"))
        dram = ctx.enter_context(tc.tile_pool(name="dram", bufs=1, space="DRAM"))

        # Fused-weight matmuls allow only ONE sync wait at codegen. A
        # standalone LDWEIGHTS (no PSUM write, no bank hazard) absorbs
        # DMA-done waits on the PE clock so every real matmul needs at most
        # one wait. Loaded garbage weights are harmless: every real matmul
        # reloads its own.
        def pe_absorb(src_ap):
            nc.tensor.ldweights(weights=src_ap)

        # ---- ACT table warmup (Ln/Exp) so table loads overlap initial DMA
        warm = fin.tile([1, 1], f32)
        nc.vector.memset(warm, 1.0)
        nc.scalar.activation(warm, warm, AF.Ln)
        nc.scalar.activation(warm, warm, AF.Exp)

        # ---- resident inputs (controller^T split per k-chunk for fast start)
        ct_r = ct_d.ap().rearrange("(k p) b -> k p b", p=128)
        ct_sb = singles.tile([128, _KC, _BS], bf16)
        for k in range(_KC):
            nc.sync.dma_start(out=ct_sb[:, k, :], in_=ct_r[k, :, :])
        if with_bias:
            b_sb = singles.tile([1, _M], bf16)
            nc.sync.dma_start(out=b_sb, in_=b_d.ap())
            pe_absorb(b_sb[:, 0:1])
            ones1 = singles.tile([1, 128], bf16)
            nc.vector.memset(ones1, 1.0)
            pe_absorb(ones1[:, 0:1])

        # ---- per-row statistics accumulators
        dot_buf = singles.tile([128, _RT, _MC], f32)
        e2_buf = singles.tile([128, _RT, _MC], f32)
        m2_buf = singles.tile([128, _RT, _MC], f32)
        ms_buf = singles.tile([128, _RT, _MC], f32)

        mem_r = m_d.ap().rearrange("(rt p) m -> rt p m", p=128)
        w_r = w_d.ap().rearrange("(k p) (mc s) -> p k mc s", p=128, s=512)

        # ---- main streaming loop
        for mc in range(_MC):
            w_sb = wpool.tile([128, _KC, 512], bf16)
            if mc == 0:
                for k in range(_KC):
                    nc.sync.dma_start(out=w_sb[:, k, :], in_=w_r[:, k, mc, :])
                    pe_absorb(w_sb[:, k, 0:1])
                    pe_absorb(ct_sb[:, k, 0:1])
            else:
                nc.sync.dma_start(out=w_sb, in_=w_r[:, :, mc, :])
                pe_absorb(w_sb[:, 0, 0:1])
            for rt in range(_RT):
                m_t = mpool.tile([128, 512], bf16)
                nc.sync.dma_start(out=m_t, in_=mem_r[rt, :, bass.ts(mc, 512)])

                ps = psum.tile([128, 512], f32)
                if with_bias:
                    nc.tensor.matmul(
                        ps, ones1, b_sb[:, bass.ts(mc, 512)],
                        start=True, stop=False,
                    )
                for k in range(_KC):
                    nc.tensor.matmul(
                        ps,
                        ct_sb[:, k, bass.ts(rt, 128)],
                        w_sb[:, k, :],
                        start=(k == 0 and not with_bias),
                        stop=(k == _KC - 1),
                    )

                e_t = epool.tile([128, 512], bf16)
                nc.scalar.activation(e_t, ps, AF.Exp)
                scr2 = spool.tile([128, 512], bf16, tag="scr2")
                nc.scalar.activation(
                    scr2, ps, AF.Exp, scale=2.0,
                    accum_out=e2_buf[:, rt, mc : mc + 1],
                )
                scr3 = spool.tile([128, 512], bf16, tag="scr3")
                nc.vector._custom_dve(
                    TENSOR_TENSOR_REDUCE, out=scr3, in0=e_t, in1=m_t,
                    s0=0.0, s1=1.0,
                    accum_out=dot_buf[:, rt, mc : mc + 1],
                )
                scr4 = spool.tile([128, 512], bf16, tag="scr4")
                nc.vector._custom_dve(
                    TENSOR_TENSOR_REDUCE, out=scr4, in0=m_t, in1=m_t,
                    s0=0.0, s1=1.0,
                    accum_out=m2_buf[:, rt, mc : mc + 1],
                )
                # rowsum(mem): alternate engines to balance ACT vs DVE
                scr5 = spool.tile([128, 512], bf16, tag="scr5")
                if (mc + rt) % 2 == 0:
                    nc.scalar.activation(
                        scr5, m_t, AF.Identity,
                        accum_out=ms_buf[:, rt, mc : mc + 1],
                    )
                else:
                    nc.vector.tensor_scalar(
                        out=scr5, in0=m_t, scalar1=1.0, scalar2=0.0,
                        op0=OP.mult, op1=OP.add,
                        accum_out=ms_buf[:, rt, mc : mc + 1],
                    )

        # ---- per-row finalization
        dot_r = fin.tile([128, _RT], f32)
        nc.vector.tensor_reduce(
            out=dot_r, in_=dot_buf, axis=mybir.AxisListType.X, op=OP.add
        )
        e2_r = fin.tile([128, _RT], f32)
        nc.vector.tensor_reduce(
            out=e2_r, in_=e2_buf, axis=mybir.AxisListType.X, op=OP.add
        )
        msum_r = fin.tile([128, _RT], f32)
        nc.vector.tensor_reduce(
            out=msum_r, in_=ms_buf, axis=mybir.AxisListType.X, op=OP.add
        )
        m2_r = fin.tile([128, _RT], f32)
        nc.vector.tensor_reduce(
            out=m2_r, in_=m2_buf, axis=mybir.AxisListType.X, op=OP.add
        )

        # sims = -dot * rsqrt(e2 * m2); exp via ACT (rsqrt = exp(-0.5 * ln))
        u = fin.tile([128, _RT], f32)
        nc.vector.tensor_tensor(out=u, in0=e2_r, in1=m2_r, op=OP.mult)
        lnu = fin.tile([128, _RT], f32)
        nc.scalar.activation(lnu, u, AF.Ln)
        rs = fin.tile([128, _RT], f32)
        nc.scalar.activation(rs, lnu, AF.Exp, scale=-0.5)
        t = fin.tile([128, _RT], f32)
        nc.vector.tensor_tensor(out=t, in0=dot_r, in1=rs, op=OP.mult)
        es = fin.tile([128, _RT], f32)
        es_sum = fin.tile([128, 1], f32)
        nc.scalar.activation(es, t, AF.Exp, scale=-1.0, accum_out=es_sum)
        wgt = fin.tile([128, _RT], f32)
        nc.vector.tensor_tensor(out=wgt, in0=es, in1=msum_r, op=OP.mult)

        # ---- global softmax denominator: sum over partitions, AllReduce
        ones128 = fin.tile([128, 1], f32)
        nc.vector.memset(ones128, 1.0)
        pe_absorb(ones128.bitcast(bf16)[:, 0:1])
        sps = psum1.tile([1, 1], f32, tag="sps")
        nc.tensor.matmul(sps, ones128, es_sum, start=True, stop=True)
        z16 = fin.tile([1, 16], f32)
        nc.vector.memset(z16, 0.0)
        nc.scalar.copy(out=z16[:, 0:1], in_=sps)

        cc_in = dram.tile([1, 16], f32)
        cc_out = dram.tile([1, 16], f32)
        nc.sync.dma_start(out=cc_in, in_=z16)
        nc.gpsimd.collective_compute(
            "AllReduce",
            OP.add,
            replica_groups=[list(range(_NC))],
            ins=[cc_in.opt()],
            outs=[cc_out.opt()],
        )
        s_b = fin.tile([128, 1], f32)
        nc.sync.dma_start(
            out=s_b,
            in_=bass.AP(tensor=cc_out.tensor, offset=cc_out.offset, ap=[[0, 128], [1, 1]]),
        )
        rinv = fin.tile([128, 1], f32)
        nc.vector.reciprocal(out=rinv, in_=s_b)
        out_sb = fin.tile([128, _RT], f32)
        nc.vector.tensor_scalar(
            out=out_sb, in0=wgt, scalar1=rinv, scalar2=None, op0=OP.mult
        )
        nc.sync.dma_start(
            out=out_d.ap().rearrange("(rt p) -> p rt", p=128), in_=out_sb
        )

    nc.compile()
    return nc


def _get_built(with_bias: bool):
    if with_bias not in _built_cache:
        _built_cache[with_bias] = _build(with_bias)
    return _built_cache[with_bias]


def _shard_inputs(memory, controller_output, W, b, with_bias):
    import ml_dtypes

    bf = ml_dtypes.bfloat16
    w_full = np.ascontiguousarray(W.astype(bf))
    b_row = np.ascontiguousarray(b.reshape(1, _M).astype(bf))
    in_maps = []
    for c in range(_NC):
        sl = slice(c * _BS, (c + 1) * _BS)
        im = {
            "ct": np.ascontiguousarray(controller_output[sl].T.astype(bf)),
            "w": w_full,
            "mem": np.ascontiguousarray(memory[sl].astype(bf)),
        }
        if with_bias:
            im["bias"] = b_row
        in_maps.append(im)
    return in_maps


def kernel(memory, controller_output, W, b):
    from concourse.bass_utils import run_bass_kernel_spmd

    memory = np.asarray(memory, dtype=np.float32)
    controller_output = np.asarray(controller_output, dtype=np.float32)
    W = np.asarray(W, dtype=np.float32)
    b = np.asarray(b, dtype=np.float32)

    with_bias = bool(np.any(b != 0.0))
    nc = _get_built(with_bias)
    in_maps = _shard_inputs(memory, controller_output, W, b, with_bias)

    res = run_bass_kernel_spmd(nc, in_maps, core_ids=list(range(_NC)))
    out = np.concatenate(
        [np.asarray(res.results[c]["out"]).reshape(-1) for c in range(_NC)]
    )
    return out.astype(np.float32)
